# revision 25
# baseline (speedup 1.0000x reference)
"""Vision-RWKV (RWKV-v4 spatial mix) encoder block on 8 Trainium2 NeuronCores.

Strategy: data-parallel over batch B=16 -> 2 batches per core, no collectives.
Layout: channel-major [c, t] on-chip (x host-transposed). The WKV recurrence
P_t = e^w * P_{t-1} + e^{k_t} v_t runs as a hardware tensor_tensor_scan along
the free (token) dim. Matmuls run in fp16 (full PE rate); scan/elementwise in
fp32. LayerNorm stats via PE ones-matmul (cross-partition reduction).

Self-contained: hardcodes B=16, T=1024, C=1024, H=W=32, 8 cores.
"""
import sys
sys.path.insert(0, "/opt/trn_rl_repo")

from contextlib import ExitStack

import numpy as np

import concourse.bacc as bacc
import concourse.tile as tile
from concourse import mybir
from concourse.bass_utils import run_bass_kernel_spmd

dt = mybir.dt
AF = mybir.ActivationFunctionType
ALU = mybir.AluOpType

B, T, C = 16, 1024, 1024
NCORES = 8
BL = B // NCORES          # batches per core
NCT = C // 128            # channel tiles (8)
TC = 512                  # token chunk
NCH = T // TC             # chunks per batch (2)
NTT = TC // 128           # token tiles per chunk (4)
EPS = 1e-5

F32, F16 = dt.float32, dt.float16

# cvec column offsets (each vector packed as [128, 8], c_tile-major columns)
MK, MV, MR, EW, EU, LW, LB = 0, 8, 16, 24, 32, 40, 48
NVEC = 56

_CACHE = {}


def _emit_dshift(nc, d16, x16, i, c):
    """d = x - qshift(x) for channel block i, chunk c, into d16[:, i*TC : (i+1)*TC].

    x16 is the full [128, 8*1024] fp16 plane (block i at cols i*1024).
    Within a block, col index = token t (0..1023). Chunk covers t in
    [c*TC, (c+1)*TC). Group g = i//2 determines the spatial shift.
    """
    g = i // 2
    xb = c * TC              # token base of this chunk
    xo = i * 1024 + xb       # col offset of this (block, chunk) in x16
    do = i * TC              # col offset in the chunk-local d16 plane
    last = c == NCH - 1
    first = c == 0

    def tt_sub(lo, hi, shift):
        # d[t] = x[t] - x[t - shift] over chunk-local t in [lo, hi)
        nc.gpsimd.tensor_tensor(
            d16[:, do + lo:do + hi],
            x16[:, xo + lo:xo + hi],
            x16[:, xo + lo - shift:xo + hi - shift],
            ALU.subtract)

    def fix(lo, hi):
        nc.vector.tensor_copy(d16[:, do + lo:do + hi], x16[:, xo + lo:xo + hi])

    def fix_strided(col):
        # copy x -> d at chunk-local t with t % 32 == col
        sr = x16[:, xo:xo + TC].rearrange("p (a q) -> p a q", q=32)
        dr = d16[:, do:do + TC].rearrange("p (a q) -> p a q", q=32)
        nc.vector.tensor_copy(dr[:, :, col:col + 1], sr[:, :, col:col + 1])

    if g == 0:        # xx = x(h, w-1), 0 at w==0  -> shift +1
        tt_sub(1 if first else 0, TC, 1)
        fix_strided(0)                                 # t % 32 == 0
    elif g == 1:      # xx = x(h, w+1), 0 at w==31 -> shift -1
        tt_sub(0, TC - 1 if last else TC, -1)
        fix_strided(31)                                # t % 32 == 31
    elif g == 2:      # xx = x(h-1, w), 0 at h==0  -> shift +32
        tt_sub(32 if first else 0, TC, 32)
        if first:
            fix(0, 32)
    else:             # xx = x(h+1, w), 0 at h==31 -> shift -32
        tt_sub(0, TC - 32 if last else TC, -32)
        if last:
            fix(TC - 32, TC)


def build():
    nc = bacc.Bacc("TRN2", target_bir_lowering=False, debug=False)

    x_d = nc.dram_tensor("x", [BL, C, T], F32, kind="ExternalInput")
    wk_d = nc.dram_tensor("wk", [C, C], F16, kind="ExternalInput")
    wv_d = nc.dram_tensor("wv", [C, C], F16, kind="ExternalInput")
    wr_d = nc.dram_tensor("wr", [C, C], F16, kind="ExternalInput")
    wo_d = nc.dram_tensor("wo", [C, C], F16, kind="ExternalInput")
    cvec_d = nc.dram_tensor("cvec", [128, NVEC], F32, kind="ExternalInput")
    out_d = nc.dram_tensor("out", [BL, T, C], F32, kind="ExternalOutput")

    with tile.TileContext(nc) as tc, ExitStack() as ctx:
        # ---- pools ----
        cons = ctx.enter_context(tc.tile_pool(name="cons", bufs=1))
        wp = ctx.enter_context(tc.tile_pool(name="wp", bufs=1))
        xin = ctx.enter_context(tc.tile_pool(name="xin", bufs=2))
        xpl = ctx.enter_context(tc.tile_pool(name="xpl", bufs=2))
        dpl = ctx.enter_context(tc.tile_pool(name="dpl", bufs=1))
        mxp = ctx.enter_context(tc.tile_pool(name="mxp", bufs=3))
        ekp = ctx.enter_context(tc.tile_pool(name="ekp", bufs=2))
        pqp = ctx.enter_context(tc.tile_pool(name="pqp", bufs=2))
        ndp = ctx.enter_context(tc.tile_pool(name="ndp", bufs=1))
        ypl = ctx.enter_context(tc.tile_pool(name="ypl", bufs=1))
        ysq = ctx.enter_context(tc.tile_pool(name="ysq", bufs=2))
        stp = ctx.enter_context(tc.tile_pool(name="stp", bufs=2))
        gtp = ctx.enter_context(tc.tile_pool(name="gtp", bufs=4))
        srp = ctx.enter_context(tc.tile_pool(name="srp", bufs=1))
        syp = ctx.enter_context(tc.tile_pool(name="syp", bufs=1))
        osg = ctx.enter_context(tc.tile_pool(name="osg", bufs=2))
        carp = ctx.enter_context(tc.tile_pool(name="carp", bufs=1))

        pp = ctx.enter_context(tc.tile_pool(name="pp", bufs=4, space="PSUM"))
        sps = ctx.enter_context(tc.tile_pool(name="sps", bufs=1, space="PSUM"))
        opl = ctx.enter_context(tc.tile_pool(name="opl", bufs=2, space="PSUM"))

        # ---- constants ----
        cvec = cons.tile([128, NVEC], F32)
        nc.sync.dma_start(cvec[:], cvec_d.ap()[:])
        ones16 = cons.tile([128, 1], F16)
        nc.vector.memset(ones16[:], 1.0 / C)
        eps_t = cons.tile([1, 1], F32)
        nc.vector.memset(eps_t[:], EPS)

        wk = wp.tile([128, NCT * C], F16, tag="wk")
        wv = wp.tile([128, NCT * C], F16, tag="wv")
        wr = wp.tile([128, NCT * C], F16, tag="wr")
        wo = wp.tile([128, NCT * C], F16, tag="wo")
        def dma_w(w_sb, w_d):
            nc.sync.dma_start(
                w_sb[:].rearrange("p (i d) -> p i d", d=C),
                w_d.ap().rearrange("(i p) d -> p i d", p=128))
        dma_w(wk, wk_d)
        dma_w(wv, wv_d)

        for b in range(BL):
            # ---- ingest x (transposed on host): [C, T] -> [128, 8*1024] fp16
            x16 = xpl.tile([128, NCT * 1024], F16, tag="x16")
            src3 = x_d.ap()[b].rearrange("(i p) t -> p i t", p=128)
            for piece in range(8):
                x32 = xin.tile([128, 1024], F32, tag="x32")
                nc.sync.dma_start(x32[:], src3[:, piece, :])
                nc.scalar.copy(
                    x16[:, piece * 1024:(piece + 1) * 1024], x32[:])

            if b == 0:
                dma_w(wr, wr_d)
                dma_w(wo, wo_d)

            carryP = carp.tile([128, NCT], F32, tag="cp")
            carryQ = carp.tile([128, NCT], F32, tag="cq")

            def emit_mixes(ch):
                d16 = dpl.tile([128, NCT * TC], F16, tag="d16")
                for i in range(NCT):
                    _emit_dshift(nc, d16, x16, i, ch)
                xk = mxp.tile([128, NCT * TC], F16, tag="mx")
                xv = mxp.tile([128, NCT * TC], F16, tag="mx")
                xr = mxp.tile([128, NCT * TC], F16, tag="mx")
                for mx, off in [(xk, MK), (xv, MV), (xr, MR)]:
                    for i in range(NCT):
                        s = slice(i * TC, (i + 1) * TC)
                        xs = slice(i * 1024 + ch * TC, i * 1024 + ch * TC + TC)
                        nc.vector.scalar_tensor_tensor(
                            mx[:, s], d16[:, s], cvec[:, off + i:off + i + 1],
                            x16[:, xs], ALU.mult, ALU.add)
                return xk, xv, xr

            mix_next = emit_mixes(0)
            for ch in range(NCH):
                xk, xv, xr = mix_next
                y16 = ypl.tile([128, NCT * TC], F16, tag="y16")

                for j in range(NCT):
                    # ---- k, v projections for output-channel block j ----
                    k_ps = pp.tile([128, TC], F32, tag="proj")
                    v_ps = pp.tile([128, TC], F32, tag="proj")
                    for w_sb, xm, ps in [(wk, xk, k_ps), (wv, xv, v_ps)]:
                        for i in range(NCT):
                            nc.tensor.matmul(
                                ps[:],
                                w_sb[:, i * C + j * 128:i * C + (j + 1) * 128],
                                xm[:, i * TC:(i + 1) * TC],
                                start=(i == 0), stop=(i == NCT - 1))

                    ek = ekp.tile([128, TC], F32, tag="ek")
                    nc.scalar.activation(ek[:], k_ps[:], AF.Exp)
                    ekv = ekp.tile([128, TC], F32, tag="ekv")
                    nc.vector.tensor_mul(ekv[:], ek[:], v_ps[:])

                    # ---- WKV scan ----
                    pbuf = pqp.tile([128, TC + 1], F32, tag="pbuf")
                    qbuf = pqp.tile([128, TC + 1], F32, tag="qbuf")
                    if ch == 0:
                        nc.vector.memset(pbuf[:, 0:1], 0.0)
                        nc.vector.memset(qbuf[:, 0:1], 0.0)
                    else:
                        nc.vector.tensor_copy(pbuf[:, 0:1], carryP[:, j:j + 1])
                        nc.vector.tensor_copy(qbuf[:, 0:1], carryQ[:, j:j + 1])
                    ewb = cvec[:, EW + j:EW + j + 1].broadcast_to([128, TC])
                    nc.vector.tensor_tensor_scan(
                        pbuf[:, 1:TC + 1], ewb, ekv[:], pbuf[:, 0:1],
                        ALU.mult, ALU.add)
                    nc.vector.tensor_tensor_scan(
                        qbuf[:, 1:TC + 1], ewb, ek[:], qbuf[:, 0:1],
                        ALU.mult, ALU.add)
                    if ch != NCH - 1:
                        nc.vector.tensor_copy(
                            carryP[:, j:j + 1], pbuf[:, TC:TC + 1])
                        nc.vector.tensor_copy(
                            carryQ[:, j:j + 1], qbuf[:, TC:TC + 1])

                    # ---- y = (P_{t-1} + e^{u+k} v) / (Q_{t-1} + e^{u+k}) ----
                    num = ndp.tile([128, TC], F32, tag="num")
                    den = ndp.tile([128, TC], F32, tag="den")
                    eu_c = cvec[:, EU + j:EU + j + 1]
                    nc.vector.scalar_tensor_tensor(
                        num[:], ekv[:], eu_c, pbuf[:, 0:TC], ALU.mult, ALU.add)
                    nc.vector.scalar_tensor_tensor(
                        den[:], ek[:], eu_c, qbuf[:, 0:TC], ALU.mult, ALU.add)
                    nc.vector.reciprocal_approx_fast(den[:], den[:])
                    yb = y16[:, j * TC:(j + 1) * TC]
                    nc.vector.tensor_mul(yb, num[:], den[:])

                    # ---- LN stats: lhsT ones = 1/C so psum rows are mu, E[y2]
                    ys = ysq.tile([128, TC], F16, tag="ysq")
                    nc.scalar.square(ys[:], yb)
                    if j == 0:
                        st_ps = sps.tile([1, 2 * TC], F32, tag="stat")
                    nc.tensor.matmul(st_ps[:, 0:TC], ones16[:], yb,
                                     start=(j == 0), stop=(j == NCT - 1))
                    nc.tensor.matmul(st_ps[:, TC:2 * TC], ones16[:], ys[:],
                                     start=(j == 0), stop=(j == NCT - 1))

                # ---- r projections + sigmoid (independent of LN) ----
                sr16 = srp.tile([128, NCT * TC], F16, tag="sr")
                for j in range(NCT):
                    r_ps = pp.tile([128, TC], F32, tag="proj")
                    for i in range(NCT):
                        nc.tensor.matmul(
                            r_ps[:],
                            wr[:, i * C + j * 128:i * C + (j + 1) * 128],
                            xr[:, i * TC:(i + 1) * TC],
                            start=(i == 0), stop=(i == NCT - 1))
                    nc.scalar.activation(sr16[:, j * TC:(j + 1) * TC],
                                         r_ps[:], AF.Sigmoid)

                # ---- next chunk's mixes fill DVE while PE runs r/out ----
                if ch + 1 < NCH:
                    mix_next = emit_mixes(ch + 1)

                # ---- LN stats post (rows on partition 0) ----
                mu16 = stp.tile([1, TC], F16, tag="mu16")
                nc.scalar.copy(mu16[:], st_ps[:, 0:TC])
                ms_t = stp.tile([1, TC], F32, tag="strow")
                nc.scalar.square(ms_t[:], st_ps[:, 0:TC])
                var_t = stp.tile([1, TC], F32, tag="strow")
                nc.vector.tensor_sub(var_t[:], st_ps[:, TC:2 * TC], ms_t[:])
                sd_t = stp.tile([1, TC], F32, tag="strow")
                nc.scalar.activation(sd_t[:], var_t[:], AF.Sqrt, bias=eps_t[:])
                nc.vector.reciprocal_approx_fast(sd_t[:], sd_t[:])
                rs16 = stp.tile([1, TC], F16, tag="rs16")
                nc.vector.tensor_scalar(rs16[:], sd_t[:], 1.0, None, ALU.mult)
                rsb = stp.tile([128, TC], F16, tag="rsb")
                nc.gpsimd.partition_broadcast(rsb[:], rs16[:])
                mub = stp.tile([128, TC], F16, tag="mub")
                nc.gpsimd.partition_broadcast(mub[:], mu16[:])

                # ---- gate: sry = sr * ((y - mu)*rstd*lnw + lnb) ----
                sry = syp.tile([128, NCT * TC], F16, tag="sry")
                for j in range(NCT):
                    srb = sr16[:, j * TC:(j + 1) * TC]
                    ya = gtp.tile([128, TC], F16, tag="gt")
                    nc.vector.tensor_sub(ya[:], y16[:, j * TC:(j + 1) * TC],
                                         mub[:])
                    ybt = gtp.tile([128, TC], F16, tag="gt")
                    nc.gpsimd.tensor_mul(ybt[:], ya[:], rsb[:])
                    yct = gtp.tile([128, TC], F16, tag="gt")
                    nc.scalar.activation(
                        yct[:], ybt[:], AF.Identity,
                        bias=cvec[:, LB + j:LB + j + 1],
                        scale=cvec[:, LW + j:LW + j + 1])
                    nc.vector.tensor_mul(sry[:, j * TC:(j + 1) * TC],
                                         yct[:], srb)

                # ---- out = sry^T @ WoT : out[t, d] tiles ----
                for m in range(NTT):
                    og = osg.tile([128, C], F32, tag="ostg")
                    for dh in range(2):
                        o_ps = opl.tile([128, TC], F32, tag="oproj")
                        for i in range(NCT):
                            nc.tensor.matmul(
                                o_ps[:],
                                sry[:, i * TC + m * 128:i * TC + (m + 1) * 128],
                                wo[:, i * C + dh * TC:i * C + (dh + 1) * TC],
                                start=(i == 0), stop=(i == NCT - 1))
                        nc.scalar.copy(og[:, dh * TC:(dh + 1) * TC], o_ps[:])
                    trow = (ch * NTT + m) * 128
                    nc.sync.dma_start(
                        out_d.ap()[b, trow:trow + 128, :], og[:])

    nc.compile()
    return nc


def _pack(v):
    return np.ascontiguousarray(v.reshape(NCT, 128).T.astype(np.float32))


def kernel(x, Wk, Wv, Wr, Wo, ln_w, ln_b, spatial_decay, spatial_first,
           mix_k, mix_v, mix_r, H, W):
    x = np.asarray(x, dtype=np.float32)
    assert int(H) == 32 and int(W) == 32 and x.shape == (B, T, C)

    if "nc" not in _CACHE:
        _CACHE["nc"] = build()
    nc = _CACHE["nc"]

    w_eff = -np.exp(np.asarray(spatial_decay, np.float64) / T)
    u_eff = np.asarray(spatial_first, np.float64) / T
    cvec = np.concatenate([
        _pack(np.asarray(mix_k, np.float32) - 1.0),
        _pack(np.asarray(mix_v, np.float32) - 1.0),
        _pack(np.asarray(mix_r, np.float32) - 1.0),
        _pack(np.exp(w_eff).astype(np.float32)),
        _pack(np.exp(u_eff).astype(np.float32)),
        _pack(np.asarray(ln_w, np.float32)),
        _pack(np.asarray(ln_b, np.float32)),
    ], axis=1)
    wk16 = np.ascontiguousarray(np.asarray(Wk, np.float32).T).astype(np.float16)
    wv16 = np.ascontiguousarray(np.asarray(Wv, np.float32).T).astype(np.float16)
    wr16 = np.ascontiguousarray(np.asarray(Wr, np.float32).T).astype(np.float16)
    wo16 = np.ascontiguousarray(np.asarray(Wo, np.float32).T).astype(np.float16)
    x_t = np.ascontiguousarray(x.transpose(0, 2, 1))

    in_maps = []
    for c in range(NCORES):
        in_maps.append({
            "x": x_t[c * BL:(c + 1) * BL],
            "wk": wk16, "wv": wv16, "wr": wr16, "wo": wo16,
            "cvec": cvec,
        })
    res = run_bass_kernel_spmd(nc, in_maps, core_ids=list(range(NCORES)))
    out = np.concatenate([res.results[c]["out"] for c in range(NCORES)], axis=0)
    return out.astype(np.float32)
